# revision 48
# baseline (speedup 1.0000x reference)
"""GCN layer kernel for 8 Trainium2 NeuronCores (Bass/Tile).

out[d] = sum_{e: dst[e]==d} vals[e] * (embeds @ W)[src[e]]

Strategy (dst-sharding, no collectives, pure streaming):
  - Destinations sharded across 8 cores (12500 each). W is linear, so
    aggregate in the embedding domain first:
      out[d] = (sum_e val_e * embeds[src_e]) @ W.
  - Host groups each core's dsts into BINS (<= nd dst slots, <= 128
    edges) under a bin profile shared by all cores (SPMD); BPB bins of
    widths BIN_NDS (summing to 128) form a BLOCK of 128 dst slots.
    Each bin is one 128-edge-slot chunk.
  - Host lays out ONE dense fused HBM slab per core (fp8 e3m4), with
    per-segment regions [G cols | S cols]:
      G [128, K*128]: slot-major gathered source rows,
      S [128, sum(nd)]: per-bin scaled one-hot scatter tiles
        S[e, dstoff] = val_e (nd columns per bin, not 128 - this is
        the big win over a full one-hot: scatter bytes drop 6x).
    The device streams each segment with a single big HWDGE DMA (no
    dma_gather - Q7 descriptor generation was the original 88%-busy
    bottleneck; and few big DMAs - each extra DMA costs ~0.5us of
    issue/sem-lane serialization).
  - TensorE: per block one PSUM accumulation group; bin w's matmuls
    write the disjoint column window [poff_w, poff_w+nd_w): the
    start=True of the block's first matmul marks the whole 2KB PSUM
    zero region pending, each window's first write zero-fills its own
    columns, later writes accumulate (per-element has_written).
  - Finale per 4 blocks: psum -> SBUF agg (fp16), one stationary W
    matmul [128, 512], copy, DMA to a transposed bf16 output
    [128, NB*128]; host un-transposes and un-permutes.
"""

import os
import ml_dtypes
import numpy as np

import concourse.bacc as bacc
import concourse.bass as bass
import concourse.mybir as mybir
import concourse.tile as tile
from concourse.bass_utils import run_bass_kernel_spmd

P = 128          # partitions / dst slots per block / edge slots per chunk
D = 128          # feature dim
N_CORES = 8
N_NODES = 100000
R_PER_CORE = N_NODES // N_CORES

SEG = 96         # max chunks per streamed segment
FIN_B = 4        # blocks per finale matmul (N = FIN_B*128 <= 512, one bank)
OUT_GRP = 7      # finale groups per output DMA


def _segments(K):
    """Big middle segments (few DMAs), small tail segments so the
    drain after the last byte is short."""
    sizes = []
    rem = K - 78
    while rem > 96:
        sizes.append(96)
        rem -= 96
    if rem > 0:
        sizes.append(rem)
    sizes += [48, 30]
    assert sum(sizes) == K
    segs = []
    c = 0
    for n in sizes:
        segs.append((c, c + n))
        c += n
    return segs

BIN_NDS = [22, 22, 22, 21, 21, 20]   # bin widths per block, sum = 128
BPB = len(BIN_NDS)
assert sum(BIN_NDS) == P
BIN_POFF = np.concatenate([[0], np.cumsum(BIN_NDS)])[:-1]

_DT = {
    "bf16": (mybir.dt.bfloat16, ml_dtypes.bfloat16),
    "fp8e4": (mybir.dt.float8e4, ml_dtypes.float8_e4m3),
    "fp8e3": (mybir.dt.float8e3, ml_dtypes.float8_e3m4),
}
G_DT = os.environ.get("GCN_G_DT", "fp8e3")
P_DT = os.environ.get("GCN_P_DT", "fp8e3")
OUT_BF16 = os.environ.get("GCN_OUT_BF16", "1") == "1"

_program_cache = {}


# ----------------------------------------------------------------- builder
def build_program(NB, n_cores=N_CORES):
    K = NB * BPB
    f32 = mybir.dt.float32
    bf16 = mybir.dt.bfloat16
    f16 = mybir.dt.float16
    g_dt = _DT[G_DT][0]
    p_dt = _DT[P_DT][0]
    assert g_dt == p_dt, "fused G|S segments need one dtype"
    o_dt = bf16 if OUT_BF16 else f32

    # S column layout: bin k has BIN_NDS[k % BPB] columns
    nds = np.array([BIN_NDS[k % BPB] for k in range(K)], np.int64)
    scol = np.concatenate([[0], np.cumsum(nds)])
    SCOLS = int(scol[-1])
    segs = _segments(K)
    seg_of_chunk = np.empty(K, np.int64)
    for si, (c0, c1) in enumerate(segs):
        seg_of_chunk[c0:c1] = si
    # fused per-segment layout: [G cols | S cols] per segment
    seg_w = max(
        (c1 - c0) * D + int(scol[c1] - scol[c0]) for c0, c1 in segs
    )
    seg_base = []
    tot = 0
    for c0, c1 in segs:
        seg_base.append(tot)
        tot += (c1 - c0) * D + int(scol[c1] - scol[c0])

    nc = bacc.Bacc(
        "TRN2", target_bir_lowering=False, debug=False, num_devices=n_cores
    )
    gsl = nc.dram_tensor("gsrc", [P, tot], g_dt, kind="ExternalInput").ap()
    wgt = nc.dram_tensor("weight", [P, D], f16, kind="ExternalInput").ap()
    out = nc.dram_tensor("out", [P, NB * P], o_dt, kind="ExternalOutput").ap()

    with tile.TileContext(nc) as tc:
        with (
            tc.tile_pool(name="const", bufs=1) as cpool,
            tc.tile_pool(name="gpool", bufs=7) as gpool,
            tc.tile_pool(name="apool", bufs=4) as apool,
            tc.tile_pool(name="opool", bufs=2) as opool,
            tc.tile_pool(name="psa", bufs=5, space="PSUM") as psa,
            tc.tile_pool(name="pso", bufs=2, space="PSUM") as pso,
        ):
            w_s = cpool.tile([P, D], f16, tag="w")
            nc.scalar.dma_start(out=w_s[:], in_=wgt[:])

            g_tiles = {}

            def ensure_seg(s):
                if s in g_tiles:
                    return
                c0, c1 = segs[s]
                n = (c1 - c0) * D + int(scol[c1] - scol[c0])
                gt = gpool.tile([P, seg_w], g_dt, tag="g")
                nc.sync.dma_start(
                    out=gt[:, :n], in_=gsl[:, seg_base[s] : seg_base[s] + n]
                )
                g_tiles[s] = (gt, c0, (c1 - c0) * D - int(scol[c0]))

            ps_a = None
            o_s = None
            o_base = 0
            ngroups = -(-NB // FIN_B)
            gstart = 0
            for b in range(NB):
                gb = b % FIN_B
                if gb == 0:
                    ps_a = psa.tile([P, FIN_B * P], f32, tag="psa")
                    gstart = b
                last_grp = b == NB - 1 or gb == FIN_B - 1
                for w in range(BPB):
                    k = b * BPB + w
                    s = int(seg_of_chunk[k])
                    ensure_seg(s)
                    gt, c0, s_off = g_tiles[s]
                    off = k - c0
                    nd = BIN_NDS[w]
                    po = gb * P + int(BIN_POFF[w])
                    sc = s_off + int(scol[k])
                    nc.tensor.matmul(
                        out=ps_a[:, po : po + nd],
                        lhsT=gt[:, off * D : (off + 1) * D],
                        rhs=gt[:, sc : sc + nd],
                        start=(gb == 0 and w == 0),
                        stop=(last_grp and w == BPB - 1),
                        skip_group_check=True,
                    )
                if last_grp:
                    n = (gb + 1) * P
                    g = b // FIN_B
                    agg_cur = apool.tile([P, FIN_B * P], f16, tag="agg")
                    h = n // 2
                    nc.vector.tensor_copy(out=agg_cur[:, :h], in_=ps_a[:, :h])
                    nc.scalar.copy(out=agg_cur[:, h:n], in_=ps_a[:, h:n])
                    ps_o = pso.tile([P, FIN_B * P], f32, tag="pso")
                    nc.tensor.matmul(
                        out=ps_o[:, :n],
                        lhsT=w_s[:],
                        rhs=agg_cur[:, :n],
                        start=True,
                        stop=True,
                    )
                    if o_s is None:
                        o_s = opool.tile([P, OUT_GRP * FIN_B * P], o_dt, tag="out")
                        o_base = gstart * P
                    oo = gstart * P - o_base
                    nc.scalar.copy(out=o_s[:, oo : oo + h], in_=ps_o[:, :h])
                    nc.vector.tensor_copy(
                        out=o_s[:, oo + h : oo + n], in_=ps_o[:, h:n]
                    )
                    if (
                        g % OUT_GRP == OUT_GRP - 1
                        or b == NB - 1
                        or g == ngroups - 2
                    ):
                        # sync queue is idle after the input issues; keep
                        # the out-flush issue cost off the busy ACT queue
                        nc.sync.dma_start(
                            out=out[:, o_base : o_base + oo + n],
                            in_=o_s[:, : oo + n],
                        )
                        o_s = None

    nc.compile()
    return nc


# ----------------------------------------------------------- preprocessing
def _pack_core(deg, NB):
    """Assign local dsts to bins: bin i (i = block*BPB + w) takes
    <= BIN_NDS[w] dsts totaling <= 128 edges. Vectorized best-fit,
    big dsts first. Returns (bin_of, idx_in_bin) per dst."""
    nbins = NB * BPB
    ndcap = np.array([BIN_NDS[i % BPB] for i in range(nbins)], np.int64)
    rem = np.full(nbins, P, np.int64)    # remaining edge slots
    cnt = np.zeros(nbins, np.int64)
    Rn = deg.shape[0]
    bin_of = np.empty(Rn, np.int32)
    idx_of = np.empty(Rn, np.int32)
    order = np.argsort(-deg, kind="stable")
    for d in order:
        dv = deg[d]
        after = rem - dv
        feas = (cnt < ndcap) & (after >= 0)
        if not feas.any():
            raise RuntimeError("packing failed")
        score = np.where(feas, after, -1)
        b = int(score.argmax())
        bin_of[d] = b
        idx_of[d] = cnt[b]
        cnt[b] += 1
        rem[b] -= dv
    return bin_of, idx_of


def preprocess(embeds, weight, edge_index, edge_vals, n_cores=N_CORES):
    n_nodes = embeds.shape[0]
    Rn = n_nodes // n_cores
    dst = edge_index[0].astype(np.int64)
    src = edge_index[1].astype(np.int64)
    vals = edge_vals.astype(np.float32)
    core = dst // Rn
    assert core.max() < n_cores

    per_core = []
    degs = np.zeros((n_cores, Rn), np.int64)
    for c in range(n_cores):
        m = core == c
        ld = dst[m] - c * Rn
        per_core.append((ld, src[m], vals[m]))
        np.add.at(degs[c], ld, 1)

    kmax = int(degs.sum(1).max())
    NB = -(-int(np.ceil(kmax * 1.065)) // (BPB * P))
    packs = None
    for _ in range(6):
        try:
            packs = [_pack_core(degs[c], NB) for c in range(n_cores)]
            break
        except RuntimeError:
            NB += 2
    if packs is None:
        raise RuntimeError("bin packing failed after escalation")

    K = NB * BPB
    nds = np.array([BIN_NDS[k % BPB] for k in range(K)], np.int64)
    scol = np.concatenate([[0], np.cumsum(nds)])
    SCOLS = int(scol[-1])

    g_np = _DT[G_DT][1]
    p_np = _DT[P_DT][1]
    emb_g = np.ascontiguousarray(embeds.astype(g_np))
    w_h = np.ascontiguousarray(weight.astype(np.float16))

    in_maps, rowmaps = [], []
    for c in range(n_cores):
        ld, lsrc, lval = per_core[c]
        bin_of, idx_of = packs[c]
        eb = bin_of[ld]                      # bin per edge
        order = np.argsort(eb, kind="stable")
        eb_s = eb[order]
        src_s = lsrc[order]
        val_s = lval[order]
        dof_e = idx_of[ld][order].astype(np.int64)   # col within bin
        n_per = np.bincount(eb_s, minlength=K)
        start = np.concatenate([[0], np.cumsum(n_per)])[:-1]
        slot = np.arange(len(eb_s)) - start[eb_s]    # edge slot in chunk
        assert (slot < P).all()

        # G part: [slot, bin*D + f] = embeds[src, f]
        srcs = np.zeros(K * P, np.int64)
        srcs[eb_s * P + slot] = src_s
        gl = emb_g[srcs]
        gsl_h = gl.reshape(K, P, D).transpose(1, 0, 2).reshape(P, K * D)

        # S part: [slot, scol[bin] + dstoff] = val (column-sparse layout)
        sl = np.zeros((P, SCOLS), np.float32)
        sl[slot, scol[eb_s] + dof_e] = val_s
        ssl_h = sl.astype(g_np)

        # fused per-segment slab: [G cols | S cols] per segment
        parts = []
        for c0, c1 in _segments(K):
            parts.append(gsl_h[:, c0 * D : c1 * D])
            parts.append(ssl_h[:, scol[c0] : scol[c1]])
        gs = np.ascontiguousarray(np.concatenate(parts, axis=1))

        in_maps.append({"gsrc": gs, "weight": w_h})
        # dst -> (block, col within block)
        blk = bin_of // BPB
        col = BIN_POFF[bin_of % BPB] + idx_of
        rowmaps.append(blk.astype(np.int64) * P + col.astype(np.int64))

    return in_maps, rowmaps, NB, Rn


# ------------------------------------------------------------------ kernel
def kernel(embeds, weight, edge_index, edge_vals):
    embeds = np.asarray(embeds, dtype=np.float32)
    weight = np.asarray(weight, dtype=np.float32)
    edge_index = np.asarray(edge_index)
    edge_vals = np.asarray(edge_vals, dtype=np.float32)

    in_maps, rowmaps, NB, Rn = preprocess(embeds, weight, edge_index, edge_vals)

    key = (G_DT, P_DT, OUT_BF16, NB)
    if key not in _program_cache:
        _program_cache[key] = build_program(NB)
    nc = _program_cache[key]

    want_trace = os.environ.get("GCN_TRACE") == "1"
    res = run_bass_kernel_spmd(
        nc,
        in_maps,
        core_ids=list(range(N_CORES)),
        trace=want_trace,
    )
    if want_trace:
        kernel.last_exec_time_ns = res.exec_time_ns
        kernel.last_results = res

    n_nodes = embeds.shape[0]
    out = np.empty((n_nodes, D), np.float32)
    for c in range(N_CORES):
        o = np.asarray(res.results[c]["out"]).astype(np.float32)
        out[c * Rn : (c + 1) * Rn] = o.T[rowmaps[c]]
    return out


# revision 49
# speedup vs baseline: 1.0466x; 1.0466x over previous
"""GCN layer kernel for 8 Trainium2 NeuronCores (Bass/Tile).

out[d] = sum_{e: dst[e]==d} vals[e] * (embeds @ W)[src[e]]

Strategy (dst-sharding, no collectives, pure streaming):
  - Destinations sharded across 8 cores (12500 each). W is linear, so
    aggregate in the embedding domain first:
      out[d] = (sum_e val_e * embeds[src_e]) @ W.
  - Host groups each core's dsts into BINS (<= nd dst slots, <= 128
    edges) under a bin profile shared by all cores (SPMD); BPB bins of
    widths BIN_NDS (summing to 128) form a BLOCK of 128 dst slots.
    Each bin is one 128-edge-slot chunk.
  - Host lays out ONE dense fused HBM slab per core (fp8 e3m4), with
    per-segment regions [G cols | S cols]:
      G [128, K*128]: slot-major gathered source rows,
      S [128, sum(nd)]: per-bin scaled one-hot scatter tiles
        S[e, dstoff] = val_e (nd columns per bin, not 128 - this is
        the big win over a full one-hot: scatter bytes drop 6x).
    The device streams each segment with a single big HWDGE DMA (no
    dma_gather - Q7 descriptor generation was the original 88%-busy
    bottleneck; and few big DMAs - each extra DMA costs ~0.5us of
    issue/sem-lane serialization).
  - TensorE: per block one PSUM accumulation group; bin w's matmuls
    write the disjoint column window [poff_w, poff_w+nd_w): the
    start=True of the block's first matmul marks the whole 2KB PSUM
    zero region pending, each window's first write zero-fills its own
    columns, later writes accumulate (per-element has_written).
  - Finale per 4 blocks: psum -> SBUF agg (fp16), one stationary W
    matmul [128, 512], copy, DMA to a transposed bf16 output
    [128, NB*128]; host un-transposes and un-permutes.
"""

import os
import ml_dtypes
import numpy as np

import concourse.bacc as bacc
import concourse.bass as bass
import concourse.mybir as mybir
import concourse.tile as tile
from concourse.bass_utils import run_bass_kernel_spmd

P = 128          # partitions / dst slots per block / edge slots per chunk
D = 128          # feature dim
N_CORES = 8
N_NODES = 100000
R_PER_CORE = N_NODES // N_CORES

SEG = 96         # max chunks per streamed segment
FIN_B = 4        # blocks per finale matmul (N = FIN_B*128 <= 512, one bank)
OUT_GRP = 7      # finale groups per output DMA


def _segments(K):
    """Big middle segments (few DMAs), small tail segments so the
    drain after the last byte is short."""
    sizes = []
    rem = K - 78
    while rem > 96:
        sizes.append(96)
        rem -= 96
    if rem > 0:
        sizes.append(rem)
    sizes += [48, 30]
    assert sum(sizes) == K
    segs = []
    c = 0
    for n in sizes:
        segs.append((c, c + n))
        c += n
    return segs

BIN_NDS = [22, 22, 22, 21, 21, 20]   # bin widths per block, sum = 128
BPB = len(BIN_NDS)
assert sum(BIN_NDS) == P
BIN_POFF = np.concatenate([[0], np.cumsum(BIN_NDS)])[:-1]

_DT = {
    "bf16": (mybir.dt.bfloat16, ml_dtypes.bfloat16),
    "fp8e4": (mybir.dt.float8e4, ml_dtypes.float8_e4m3),
    "fp8e3": (mybir.dt.float8e3, ml_dtypes.float8_e3m4),
}
G_DT = os.environ.get("GCN_G_DT", "fp8e3")
P_DT = os.environ.get("GCN_P_DT", "fp8e3")
OUT_BF16 = os.environ.get("GCN_OUT_BF16", "1") == "1"

_program_cache = {}


# ----------------------------------------------------------------- builder
def build_program(NB, n_cores=N_CORES):
    K = NB * BPB
    f32 = mybir.dt.float32
    bf16 = mybir.dt.bfloat16
    f16 = mybir.dt.float16
    g_dt = _DT[G_DT][0]
    p_dt = _DT[P_DT][0]
    assert g_dt == p_dt, "fused G|S segments need one dtype"
    o_dt = bf16 if OUT_BF16 else f32

    # S column layout: bin k has BIN_NDS[k % BPB] columns
    nds = np.array([BIN_NDS[k % BPB] for k in range(K)], np.int64)
    scol = np.concatenate([[0], np.cumsum(nds)])
    SCOLS = int(scol[-1])
    segs = _segments(K)
    seg_of_chunk = np.empty(K, np.int64)
    for si, (c0, c1) in enumerate(segs):
        seg_of_chunk[c0:c1] = si
    # fused per-segment layout: [G cols | S cols] per segment
    seg_w = max(
        (c1 - c0) * D + int(scol[c1] - scol[c0]) for c0, c1 in segs
    )
    seg_base = []
    tot = 0
    for c0, c1 in segs:
        seg_base.append(tot)
        tot += (c1 - c0) * D + int(scol[c1] - scol[c0])

    nc = bacc.Bacc(
        "TRN2", target_bir_lowering=False, debug=False, num_devices=n_cores
    )
    gsl = nc.dram_tensor("gsrc", [P, tot], g_dt, kind="ExternalInput").ap()
    wgt = nc.dram_tensor("weight", [P, D], f16, kind="ExternalInput").ap()
    out = nc.dram_tensor("out", [P, NB * P], o_dt, kind="ExternalOutput").ap()

    with tile.TileContext(nc) as tc:
        with (
            tc.tile_pool(name="const", bufs=1) as cpool,
            tc.tile_pool(name="gpool", bufs=7) as gpool,
            tc.tile_pool(name="apool", bufs=4) as apool,
            tc.tile_pool(name="opool", bufs=2) as opool,
            tc.tile_pool(name="psa", bufs=5, space="PSUM") as psa,
            tc.tile_pool(name="pso", bufs=2, space="PSUM") as pso,
        ):
            w_s = cpool.tile([P, D], f16, tag="w")
            nc.scalar.dma_start(out=w_s[:], in_=wgt[:])

            g_tiles = {}

            def ensure_seg(s):
                if s in g_tiles:
                    return
                c0, c1 = segs[s]
                n = (c1 - c0) * D + int(scol[c1] - scol[c0])
                gt = gpool.tile([P, seg_w], g_dt, tag="g")
                nc.sync.dma_start(
                    out=gt[:, :n], in_=gsl[:, seg_base[s] : seg_base[s] + n]
                )
                g_tiles[s] = (gt, c0, (c1 - c0) * D - int(scol[c0]))

            ps_a = None
            o_s = None
            o_base = 0
            ngroups = -(-NB // FIN_B)
            gstart = 0
            for b in range(NB):
                gb = b % FIN_B
                if gb == 0:
                    ps_a = psa.tile([P, FIN_B * P], f32, tag="psa")
                    gstart = b
                last_grp = b == NB - 1 or gb == FIN_B - 1
                for w in range(BPB):
                    k = b * BPB + w
                    s = int(seg_of_chunk[k])
                    ensure_seg(s)
                    gt, c0, s_off = g_tiles[s]
                    off = k - c0
                    nd = BIN_NDS[w]
                    po = gb * P + int(BIN_POFF[w])
                    sc = s_off + int(scol[k])
                    nc.tensor.matmul(
                        out=ps_a[:, po : po + nd],
                        lhsT=gt[:, off * D : (off + 1) * D],
                        rhs=gt[:, sc : sc + nd],
                        start=(gb == 0 and w == 0),
                        stop=(last_grp and w == BPB - 1),
                        skip_group_check=True,
                    )
                if last_grp:
                    n = (gb + 1) * P
                    g = b // FIN_B
                    agg_cur = apool.tile([P, FIN_B * P], f16, tag="agg")
                    h = n // 2
                    nc.vector.tensor_copy(out=agg_cur[:, :h], in_=ps_a[:, :h])
                    nc.scalar.copy(out=agg_cur[:, h:n], in_=ps_a[:, h:n])
                    ps_o = pso.tile([P, FIN_B * P], f32, tag="pso")
                    nc.tensor.matmul(
                        out=ps_o[:, :n],
                        lhsT=w_s[:],
                        rhs=agg_cur[:, :n],
                        start=True,
                        stop=True,
                    )
                    if o_s is None:
                        o_s = opool.tile([P, OUT_GRP * FIN_B * P], o_dt, tag="out")
                        o_base = gstart * P
                    oo = gstart * P - o_base
                    nc.scalar.copy(out=o_s[:, oo : oo + h], in_=ps_o[:, :h])
                    nc.vector.tensor_copy(
                        out=o_s[:, oo + h : oo + n], in_=ps_o[:, h:n]
                    )
                    if (
                        g % OUT_GRP == OUT_GRP - 1
                        or b == NB - 1
                        or g == ngroups - 2
                    ):
                        nc.scalar.dma_start(
                            out=out[:, o_base : o_base + oo + n],
                            in_=o_s[:, : oo + n],
                        )
                        o_s = None

    nc.compile()
    return nc


# ----------------------------------------------------------- preprocessing
def _pack_core(deg, NB):
    """Assign local dsts to bins: bin i (i = block*BPB + w) takes
    <= BIN_NDS[w] dsts totaling <= 128 edges. Vectorized best-fit,
    big dsts first. Returns (bin_of, idx_in_bin) per dst."""
    nbins = NB * BPB
    ndcap = np.array([BIN_NDS[i % BPB] for i in range(nbins)], np.int64)
    rem = np.full(nbins, P, np.int64)    # remaining edge slots
    cnt = np.zeros(nbins, np.int64)
    Rn = deg.shape[0]
    bin_of = np.empty(Rn, np.int32)
    idx_of = np.empty(Rn, np.int32)
    order = np.argsort(-deg, kind="stable")
    for d in order:
        dv = deg[d]
        after = rem - dv
        feas = (cnt < ndcap) & (after >= 0)
        if not feas.any():
            raise RuntimeError("packing failed")
        score = np.where(feas, after, -1)
        b = int(score.argmax())
        bin_of[d] = b
        idx_of[d] = cnt[b]
        cnt[b] += 1
        rem[b] -= dv
    return bin_of, idx_of


def preprocess(embeds, weight, edge_index, edge_vals, n_cores=N_CORES):
    n_nodes = embeds.shape[0]
    Rn = n_nodes // n_cores
    dst = edge_index[0].astype(np.int64)
    src = edge_index[1].astype(np.int64)
    vals = edge_vals.astype(np.float32)
    core = dst // Rn
    assert core.max() < n_cores

    per_core = []
    degs = np.zeros((n_cores, Rn), np.int64)
    for c in range(n_cores):
        m = core == c
        ld = dst[m] - c * Rn
        per_core.append((ld, src[m], vals[m]))
        np.add.at(degs[c], ld, 1)

    kmax = int(degs.sum(1).max())
    NB = -(-int(np.ceil(kmax * 1.065)) // (BPB * P))
    packs = None
    for _ in range(6):
        try:
            packs = [_pack_core(degs[c], NB) for c in range(n_cores)]
            break
        except RuntimeError:
            NB += 2
    if packs is None:
        raise RuntimeError("bin packing failed after escalation")

    K = NB * BPB
    nds = np.array([BIN_NDS[k % BPB] for k in range(K)], np.int64)
    scol = np.concatenate([[0], np.cumsum(nds)])
    SCOLS = int(scol[-1])

    g_np = _DT[G_DT][1]
    p_np = _DT[P_DT][1]
    emb_g = np.ascontiguousarray(embeds.astype(g_np))
    w_h = np.ascontiguousarray(weight.astype(np.float16))

    in_maps, rowmaps = [], []
    for c in range(n_cores):
        ld, lsrc, lval = per_core[c]
        bin_of, idx_of = packs[c]
        eb = bin_of[ld]                      # bin per edge
        order = np.argsort(eb, kind="stable")
        eb_s = eb[order]
        src_s = lsrc[order]
        val_s = lval[order]
        dof_e = idx_of[ld][order].astype(np.int64)   # col within bin
        n_per = np.bincount(eb_s, minlength=K)
        start = np.concatenate([[0], np.cumsum(n_per)])[:-1]
        slot = np.arange(len(eb_s)) - start[eb_s]    # edge slot in chunk
        assert (slot < P).all()

        # G part: [slot, bin*D + f] = embeds[src, f]
        srcs = np.zeros(K * P, np.int64)
        srcs[eb_s * P + slot] = src_s
        gl = emb_g[srcs]
        gsl_h = gl.reshape(K, P, D).transpose(1, 0, 2).reshape(P, K * D)

        # S part: [slot, scol[bin] + dstoff] = val (column-sparse layout)
        sl = np.zeros((P, SCOLS), np.float32)
        sl[slot, scol[eb_s] + dof_e] = val_s
        ssl_h = sl.astype(g_np)

        # fused per-segment slab: [G cols | S cols] per segment
        parts = []
        for c0, c1 in _segments(K):
            parts.append(gsl_h[:, c0 * D : c1 * D])
            parts.append(ssl_h[:, scol[c0] : scol[c1]])
        gs = np.ascontiguousarray(np.concatenate(parts, axis=1))

        in_maps.append({"gsrc": gs, "weight": w_h})
        # dst -> (block, col within block)
        blk = bin_of // BPB
        col = BIN_POFF[bin_of % BPB] + idx_of
        rowmaps.append(blk.astype(np.int64) * P + col.astype(np.int64))

    return in_maps, rowmaps, NB, Rn


# ------------------------------------------------------------------ kernel
def kernel(embeds, weight, edge_index, edge_vals):
    embeds = np.asarray(embeds, dtype=np.float32)
    weight = np.asarray(weight, dtype=np.float32)
    edge_index = np.asarray(edge_index)
    edge_vals = np.asarray(edge_vals, dtype=np.float32)

    in_maps, rowmaps, NB, Rn = preprocess(embeds, weight, edge_index, edge_vals)

    key = (G_DT, P_DT, OUT_BF16, NB)
    if key not in _program_cache:
        _program_cache[key] = build_program(NB)
    nc = _program_cache[key]

    want_trace = os.environ.get("GCN_TRACE") == "1"
    res = run_bass_kernel_spmd(
        nc,
        in_maps,
        core_ids=list(range(N_CORES)),
        trace=want_trace,
    )
    if want_trace:
        kernel.last_exec_time_ns = res.exec_time_ns
        kernel.last_results = res

    n_nodes = embeds.shape[0]
    out = np.empty((n_nodes, D), np.float32)
    for c in range(N_CORES):
        o = np.asarray(res.results[c]["out"]).astype(np.float32)
        out[c * Rn : (c + 1) * Rn] = o.T[rowmaps[c]]
    return out
